# revision 20
# baseline (speedup 1.0000x reference)
"""Talking-heads attention kernel for Trainium2, 8 NeuronCores.

Problem: B=4, N=2048, DIM=512, H=8, DH=64 talking-heads attention
(qkv proj -> per-head scores -> th1 head-mix -> softmax -> th2 head-mix
 -> attn @ v -> out proj).

Sharding: data-parallel over (batch, query-half): core c handles batch c//2,
query rows [1024*(c%2), 1024*(c%2)+1024).  Communication-free.

Device pipeline per core (all matmuls bf16 with fp32 PSUM accumulation):
  1. x -> bf16 -> xbar-transpose -> x^T; QKV projection on TensorE producing
     Q^T/K^T (head-transposed layout) and V (n-major).
  2. Per 128-query tile: per-head scores S_h = Q_h K_h^T (scale folded into
     w_q on host).
  3. Pack scores into head-interleaved tiles [(n16,h), m] via SBUF->SBUF DMA,
     then both talking-heads mixes run as full-width 128x128 block-diagonal
     matmuls on TensorE.
  4. exp on ScalarE with fused per-row accumulation (softmax denominator Z
     comes for free); the softmax division is folded into the second mix's
     stationary weights (th2[g,h]/Z row scaling), so no elementwise divide
     pass ever touches the [n,m] matrix.
  5. mix2 output is xbar-DMA-transposed to key-major A^T, attn@V runs with
     V as the stationary operand, followed by the output projection.
"""

import sys

sys.path.insert(0, "/opt/trn_rl_repo")

import numpy as np
import ml_dtypes

import concourse.bass as bass
from concourse import bacc
import concourse.mybir as mybir
import concourse.tile as tile
from concourse.bass_utils import run_bass_kernel_spmd

BF16 = mybir.dt.bfloat16
F32 = mybir.dt.float32
AF = mybir.ActivationFunctionType

B, N, DIM = 4, 2048, 512
H, DH = 8, 64
NCORES = 8
NLOC = N // 2          # query rows per core
NT = NLOC // 128       # 8 query tiles per core
MT = N // 128          # 16 key chunks
NG = 16                # n16 group size in interleaved tiles


def build_nc():
    nc = bacc.Bacc()

    x = nc.declare_dram_parameter("x", [N, DIM], F32, isOutput=False)
    xq = nc.declare_dram_parameter("xq", [NLOC, DIM], F32, isOutput=False)
    wq = nc.declare_dram_parameter("wq", [DIM, DIM], BF16, isOutput=False)
    wk = nc.declare_dram_parameter("wk", [DIM, DIM], BF16, isOutput=False)
    wv = nc.declare_dram_parameter("wv", [DIM, DIM], BF16, isOutput=False)
    wo = nc.declare_dram_parameter("wo", [DIM, DIM], BF16, isOutput=False)
    th1s = nc.declare_dram_parameter("th1s", [128, 32], F32, isOutput=False)
    t2t = nc.declare_dram_parameter("t2t", [128, 128], BF16, isOutput=False)
    # rows: bq (pre-scaled), bk, bv, bo
    bia = nc.declare_dram_parameter("bia", [1, 4 * DIM], BF16, isOutput=False)
    y = nc.declare_dram_parameter("y", [NLOC, DIM], F32, isOutput=True)

    with tile.TileContext(nc) as tc:
        with (
            tc.tile_pool(name="pw", bufs=1) as pw,      # persistent across phases
            tc.tile_pool(name="psA", bufs=3, space="PSUM") as psA,  # [128,1024] mixes
            tc.tile_pool(name="psB", bufs=2, space="PSUM") as psB,  # [128,512] scores/proj/@V
        ):
            # persistent small tensors
            wo_sb = [pw.tile([128, DIM], BF16, name=f"wo{i}", tag=f"wo{i}") for i in range(4)]
            for i in range(4):
                nc.sync.dma_start(wo_sb[i][:], wo[128 * i:128 * (i + 1), :])
            th1s_sb = pw.tile([128, 32], F32, name="th1s", tag="th1s")
            t2_sb = pw.tile([128, 128], BF16, name="t2", tag="t2")
            nc.sync.dma_start(th1s_sb[:], th1s[:])
            nc.sync.dma_start(t2_sb[:], t2t[:])
            bia_sb = pw.tile([1, 4 * DIM], BF16, name="bia", tag="bia")
            nc.sync.dma_start(bia_sb[:], bia[:])
            ones_sb = pw.tile([1, DIM], BF16, name="ones", tag="ones")
            nc.any.memset(ones_sb[:], 1.0)

            # persistent activations: th1-fused interleaved Qhat, K^T, V
            # qhat[rc][p=(h,d) in chunk rc, col (j, n16, g)] = th1[g,h] * Q^T[(h,d), n]
            #   with n = 16*j + n16 (local query index)
            qhat_sb = [pw.tile([128, NLOC * 8], BF16, name=f"qh{i}", tag=f"qh{i}")
                       for i in range(4)]
            kt_sb = [pw.tile([128, N], BF16, name=f"kt{i}", tag=f"kt{i}") for i in range(4)]
            v_sb = pw.tile([128, MT, DIM], BF16, name="v", tag="v")

            def evict(dst, src, use_act):
                if use_act:
                    nc.scalar.copy(dst, src)
                else:
                    nc.vector.tensor_copy(dst, src)

            # ================= phase A: x^T + QKV projection =================
            with tc.tile_pool(name="pxt", bufs=1) as pxt:
                qt_sb = [pxt.tile([128, NLOC], BF16, name=f"qt{i}", tag=f"qt{i}")
                         for i in range(4)]
                wq_sb = [pxt.tile([128, DIM], BF16, name=f"wq{i}", tag=f"wq{i}") for i in range(4)]
                wk_sb = [pxt.tile([128, DIM], BF16, name=f"wk{i}", tag=f"wk{i}") for i in range(4)]
                wv_sb = [pxt.tile([128, DIM], BF16, name=f"wv{i}", tag=f"wv{i}") for i in range(4)]
                for i in range(4):
                    nc.sync.dma_start(wq_sb[i][:], wq[128 * i:128 * (i + 1), :])
                    nc.sync.dma_start(wk_sb[i][:], wk[128 * i:128 * (i + 1), :])
                    nc.sync.dma_start(wv_sb[i][:], wv[128 * i:128 * (i + 1), :])

                # x -> x^T (bf16): HWDGE fp32 load, engine cast, xbar transpose
                # xt_sb[p, t, j, f]: dim = j*128+p, key row m = t*128+f
                xt_sb = pxt.tile([128, MT, 4, 128], BF16, name="xt", tag="xt")
                xqt_sb = pxt.tile([128, NT, 4, 128], BF16, name="xqt", tag="xqt")
                for t in range(MT + NT):
                    xb = pxt.tile([128, DIM], F32, name="xb", tag="xb", bufs=4)
                    if t < MT:
                        nc.sync.dma_start(xb[:], x[128 * t:128 * (t + 1), :])
                    else:
                        tq = t - MT
                        nc.sync.dma_start(xb[:], xq[128 * tq:128 * (tq + 1), :])
                    xc = pxt.tile([128, DIM], BF16, name="xc", tag="xc", bufs=4)
                    evict(xc[:], xb[:], t % 2 == 0)
                    if t < MT:
                        nc.sync.dma_start_transpose(xt_sb[:, t, :, :], xc[:])
                    else:
                        nc.sync.dma_start_transpose(xqt_sb[:, t - MT, :, :], xc[:])

                ei = 0
                for rc in range(4):            # K^T over all keys (needed first)
                    for mch in range(N // 512):
                        ps = psB.tile([128, 512], F32, name="psq", tag="pss")
                        for j in range(4):
                            rhs = xt_sb[:, 4 * mch:4 * (mch + 1), j, :]
                            nc.tensor.matmul(ps[:], wk_sb[j][:, 128 * rc:128 * (rc + 1)],
                                             rhs, start=(j == 0), stop=False)
                        nc.tensor.matmul(ps[:], bia_sb[0:1, 1 * DIM + 128 * rc:1 * DIM + 128 * (rc + 1)],
                                         ones_sb[:, 0:512], start=False, stop=True)
                        evict(kt_sb[rc][:, 512 * mch:512 * (mch + 1)], ps[:], ei % 2 == 0)
                        ei += 1
                for rc in range(4):            # Q^T own half, Qhat build interleaved
                    for nch in range(NLOC // 512):
                        ps = psB.tile([128, 512], F32, name="psq", tag="pss")
                        for j in range(4):
                            rhs = xqt_sb[:, 4 * nch:4 * (nch + 1), j, :]
                            nc.tensor.matmul(ps[:], wq_sb[j][:, 128 * rc:128 * (rc + 1)],
                                             rhs, start=(j == 0), stop=False)
                        nc.tensor.matmul(ps[:], bia_sb[0:1, 0 * DIM + 128 * rc:0 * DIM + 128 * (rc + 1)],
                                         ones_sb[:, 0:512], start=False, stop=True)
                        evict(qt_sb[rc][:, 512 * nch:512 * (nch + 1)], ps[:], ei % 2 == 0)
                        ei += 1
                    qhr = qhat_sb[rc][:].rearrange("p (j n g) -> p j n g",
                                                   n=NG, g=8)
                    qtr = qt_sb[rc][:].rearrange("p (j n) -> p j n", n=NG)
                    for g in range(8):
                        nc.vector.tensor_scalar_mul(
                            qhr[:, :, :, g], qtr, th1s_sb[:, 8 * rc + g:8 * rc + g + 1])
                def vproj(mt):
                    ps = psB.tile([128, 512], F32, name="psq", tag="pss")
                    for j in range(4):
                        nc.tensor.matmul(ps[:], xt_sb[:, mt, j, :], wv_sb[j][:],
                                         start=(j == 0), stop=False)
                    nc.tensor.matmul(ps[:], ones_sb[:, 0:128],
                                     bia_sb[0:1, 2 * DIM:3 * DIM],
                                     start=False, stop=True)
                    evict(v_sb[:, mt, :], ps[:], mt % 2 == 0)
                for mt in range(MT):
                    vproj(mt)
                vfill = []

            # ================= phase B: attention main loop =================
            # Fused design: th1 mix is folded into the score matmuls (Qhat
            # columns are th1-scaled, head-interleaved), so scores land
            # directly in interleaved PSUM: no raw-score evict, no pack DMA.
            # Pipeline: iteration i emits fscores+exp(i) woven with mix2(i-?)
            # and attn@V(i-1): PE bubbles while exp drains are filled by @V.
            with tc.tile_pool(name="pk", bufs=1) as pk:
                st = {}

                def mk_fscore(t):
                    st[t] = {"u": {}, "zz": {},
                             "at": [pk.tile([128, 8, 8, 128], BF16, name=f"at{i}",
                                            tag="at", bufs=3) for i in range(2)]}

                    def fscore(j):
                        u = pk.tile([128, N], BF16, name="u", tag="u", bufs=3)
                        zz = pk.tile([128, 68], F32, name="zz", tag="zz", bufs=3)
                        st[t]["u"][j] = u
                        st[t]["zz"][j] = zz
                        for half in range(2):
                            ps = psA.tile([128, 1024], F32, name="psm1", tag="psm")
                            for mc in range(2):
                                m0 = 1024 * half + 512 * mc
                                for rc in range(4):
                                    nc.tensor.matmul(
                                        ps[:, 512 * mc:512 * (mc + 1)],
                                        qhat_sb[rc][:, 128 * (8 * t + j):
                                                    128 * (8 * t + j) + 128],
                                        kt_sb[rc][:, m0:m0 + 512],
                                        start=(rc == 0), stop=(rc == 3))
                            nc.scalar.activation(u[:, 1024 * half:1024 * (half + 1)],
                                                 ps[:], AF.Exp,
                                                 accum_out=zz[:, half:half + 1])
                    return fscore

                def mk_m2(t):
                    at_h = st[t]["at"]

                    def m2(j):
                        u = st[t]["u"][j]
                        zz = st[t]["zz"][j]
                        nc.vector.tensor_add(zz[:, 2:3], zz[:, 0:1], zz[:, 1:2])
                        nc.vector.reciprocal(zz[:, 3:4], zz[:, 2:3])
                        l2 = zz[:, 4:68].bitcast(BF16)
                        nc.vector.tensor_scalar_mul(l2, t2_sb[:], zz[:, 3:4])
                        a = pk.tile([128, N], BF16, name="a", tag="a", bufs=2)
                        for half in range(2):
                            ps = psA.tile([128, 1024], F32, name="psm2", tag="psm")
                            for mc in range(2):
                                m0 = 1024 * half + 512 * mc
                                nc.tensor.matmul(ps[:, 512 * mc:512 * (mc + 1)],
                                                 l2, u[:, m0:m0 + 512],
                                                 start=True, stop=True)
                            evict(a[:, 1024 * half:1024 * (half + 1)], ps[:],
                                  (2 * j + half) % 4 == 0)
                        for half in range(2):
                            nc.sync.dma_start_transpose(
                                at_h[half][:, j, :, :],
                                a[:, 1024 * half:1024 * (half + 1)])
                    return m2

                def mk_av(tv):
                    at_h = st[tv]["at"]
                    otb = pk.tile([128, 4, 128], BF16, name="otb", tag="otb", bufs=2)
                    st[tv]["otb"] = otb

                    def av(rc):
                        # the two heads of the pair run in the two 64-col
                        # halves of the PE array concurrently (col tiling)
                        ps = psB.tile([128, 128], F32, name="psv", tag="pss")
                        for half in range(2):
                            for mc in range(8):
                                mchunk = half * 8 + mc
                                for gi in range(2):
                                    g = 2 * rc + gi
                                    rhs = at_h[half][:, :, mc, :].rearrange(
                                        "p j (n g) -> p j n g", g=8)[:, :, :, g]
                                    nc.tensor.matmul(
                                        ps[64 * gi:64 * (gi + 1), :],
                                        v_sb[:, mchunk, 64 * g:64 * (g + 1)],
                                        rhs, start=(mchunk == 0),
                                        stop=(mchunk == 15),
                                        tile_position=(0, 64 * gi))
                        nc.vector.tensor_copy(otb[:, rc, :], ps[:])
                    return av

                def emit_outproj(tv):
                    otb = st[tv]["otb"]
                    ps = psB.tile([128, DIM], F32, name="pso", tag="pss")
                    for rc in range(4):
                        nc.tensor.matmul(ps[:], otb[:, rc, :], wo_sb[rc][:],
                                         start=(rc == 0), stop=False)
                    nc.tensor.matmul(ps[:], ones_sb[:, 0:128],
                                     bia_sb[0:1, 3 * DIM:4 * DIM],
                                     start=False, stop=True)
                    yt = pk.tile([128, DIM], F32, name="yt", tag="yt", bufs=2)
                    nc.vector.tensor_copy(yt[:], ps[:])
                    nc.sync.dma_start(y[128 * tv:128 * (tv + 1), :], yt[:])
                    del st[tv]

                for i in range(NT + 1):
                    tm = i if i < NT else None
                    tv = i - 1 if i >= 1 else None
                    fscore = mk_fscore(tm) if tm is not None else None
                    m2 = mk_m2(tm) if tm is not None else None
                    av = mk_av(tv) if tv is not None else None
                    if tm is not None:
                        fscore(0)
                        fscore(1)
                    avq = [0, 1, 2, 3] if tv is not None else []
                    for k in range(8):
                        if tm is not None:
                            m2(k)
                            if k + 2 < 8:
                                fscore(k + 2)
                        if avq and k % 2 == 1:
                            av(avq.pop(0))
                        elif vfill and tv is None:
                            vfill.pop(0)(); vfill.pop(0)()
                            if vfill and k % 2 == 1:
                                vfill.pop(0)()
                    while avq:
                        av(avq.pop(0))
                    if tv is not None:
                        emit_outproj(tv)

    nc.compile()
    return nc


_NC_CACHE = None


def _get_nc():
    global _NC_CACHE
    if _NC_CACHE is None:
        _NC_CACHE = build_nc()
    return _NC_CACHE


def _host_prep(w_qkv, b_qkv, th1, th2, w_out, b_out):
    bf = ml_dtypes.bfloat16
    scale = DH ** -0.5
    w_qkv = np.asarray(w_qkv, dtype=np.float32)
    wq = (w_qkv[:, 0:DIM] * scale).astype(bf)
    wk = w_qkv[:, DIM:2 * DIM].astype(bf)
    wv = w_qkv[:, 2 * DIM:3 * DIM].astype(bf)
    wo = np.asarray(w_out, dtype=np.float32).astype(bf)
    th1 = np.asarray(th1, dtype=np.float32)
    th2 = np.asarray(th2, dtype=np.float32)
    # th1 spread for fused scores: th1s[p, rc*8+g] = th1[g, rc*2 + p//64]
    th1s = np.zeros((128, 32), dtype=np.float32)
    for rc in range(4):
        for g in range(8):
            for p in range(128):
                th1s[p, 8 * rc + g] = th1[g, rc * 2 + p // 64]
    # block-diag template for mix2: T[(n16,h),(n16,g)] = th2[g,h]
    t2t = np.zeros((128, 128), dtype=np.float32)
    for n16 in range(NG):
        t2t[n16 * 8:n16 * 8 + 8, n16 * 8:n16 * 8 + 8] = th2.T
    bqkv = np.asarray(b_qkv, dtype=np.float32)
    bia = np.zeros((1, 4 * DIM), dtype=np.float32)
    bia[0, 0:DIM] = bqkv[0:DIM] * scale     # q bias scaled with w_q
    bia[0, DIM:3 * DIM] = bqkv[DIM:3 * DIM]
    bia[0, 3 * DIM:] = np.asarray(b_out, dtype=np.float32)
    return (wq, wk, wv, wo, th1s, t2t.astype(bf), bia.astype(bf))


def kernel(x, w_qkv, b_qkv, th1, th2, w_out, b_out):
    x = np.asarray(x, dtype=np.float32)
    wq, wk, wv, wo, th1s, t2t, bia = _host_prep(w_qkv, b_qkv, th1, th2, w_out, b_out)
    nc = _get_nc()
    in_maps = []
    for c in range(NCORES):
        b, half = c // 2, c % 2
        in_maps.append({
            "x": np.ascontiguousarray(x[b]),
            "xq": np.ascontiguousarray(x[b, NLOC * half:NLOC * (half + 1), :]),
            "wq": wq, "wk": wk, "wv": wv, "wo": wo,
            "th1s": th1s, "t2t": t2t, "bia": bia,
        })
    res = run_bass_kernel_spmd(nc, in_maps, core_ids=list(range(NCORES)))
    out = np.empty((B, N, DIM), dtype=np.float32)
    for c in range(NCORES):
        b, half = c // 2, c % 2
        out[b, NLOC * half:NLOC * (half + 1), :] = res.results[c]["y"]
    return out


# revision 21
# speedup vs baseline: 1.0711x; 1.0711x over previous
"""Talking-heads attention kernel for Trainium2, 8 NeuronCores.

Problem: B=4, N=2048, DIM=512, H=8, DH=64 talking-heads attention
(qkv proj -> per-head scores -> th1 head-mix -> softmax -> th2 head-mix
 -> attn @ v -> out proj).

Sharding: data-parallel over (batch, query-half): core c handles batch c//2,
query rows [1024*(c%2), 1024*(c%2)+1024).  Communication-free.

Device pipeline per core (all matmuls bf16 with fp32 PSUM accumulation):
  1. x -> bf16 -> xbar-transpose -> x^T; QKV projection on TensorE producing
     Q^T/K^T (head-transposed layout) and V (n-major).
  2. Per 128-query tile: per-head scores S_h = Q_h K_h^T (scale folded into
     w_q on host).
  3. Pack scores into head-interleaved tiles [(n16,h), m] via SBUF->SBUF DMA,
     then both talking-heads mixes run as full-width 128x128 block-diagonal
     matmuls on TensorE.
  4. exp on ScalarE with fused per-row accumulation (softmax denominator Z
     comes for free); the softmax division is folded into the second mix's
     stationary weights (th2[g,h]/Z row scaling), so no elementwise divide
     pass ever touches the [n,m] matrix.
  5. mix2 output is xbar-DMA-transposed to key-major A^T, attn@V runs with
     V as the stationary operand, followed by the output projection.
"""

import sys

sys.path.insert(0, "/opt/trn_rl_repo")

import numpy as np
import ml_dtypes

import concourse.bass as bass
from concourse import bacc
import concourse.mybir as mybir
import concourse.tile as tile
from concourse.bass_utils import run_bass_kernel_spmd

BF16 = mybir.dt.bfloat16
F32 = mybir.dt.float32
AF = mybir.ActivationFunctionType

B, N, DIM = 4, 2048, 512
H, DH = 8, 64
NCORES = 8
NLOC = N // 2          # query rows per core
NT = NLOC // 128       # 8 query tiles per core
MT = N // 128          # 16 key chunks
NG = 16                # n16 group size in interleaved tiles


def build_nc():
    nc = bacc.Bacc()

    x = nc.declare_dram_parameter("x", [N, DIM], F32, isOutput=False)
    xq = nc.declare_dram_parameter("xq", [NLOC, DIM], F32, isOutput=False)
    wq = nc.declare_dram_parameter("wq", [DIM, DIM], BF16, isOutput=False)
    wk = nc.declare_dram_parameter("wk", [DIM, DIM], BF16, isOutput=False)
    wv = nc.declare_dram_parameter("wv", [DIM, DIM], BF16, isOutput=False)
    wo = nc.declare_dram_parameter("wo", [DIM, DIM], BF16, isOutput=False)
    th1s = nc.declare_dram_parameter("th1s", [128, 32], F32, isOutput=False)
    t2t = nc.declare_dram_parameter("t2t", [128, 128], BF16, isOutput=False)
    # rows: bq (pre-scaled), bk, bv, bo
    bia = nc.declare_dram_parameter("bia", [1, 4 * DIM], BF16, isOutput=False)
    y = nc.declare_dram_parameter("y", [NLOC, DIM], F32, isOutput=True)

    with tile.TileContext(nc) as tc:
        with (
            tc.tile_pool(name="pw", bufs=1) as pw,      # persistent across phases
            tc.tile_pool(name="psA", bufs=3, space="PSUM") as psA,  # [128,1024] mixes
            tc.tile_pool(name="psB", bufs=2, space="PSUM") as psB,  # [128,512] scores/proj/@V
        ):
            # persistent small tensors
            wo_sb = [pw.tile([128, DIM], BF16, name=f"wo{i}", tag=f"wo{i}") for i in range(4)]
            for i in range(4):
                nc.sync.dma_start(wo_sb[i][:], wo[128 * i:128 * (i + 1), :])
            th1s_sb = pw.tile([128, 32], F32, name="th1s", tag="th1s")
            t2_sb = pw.tile([128, 128], BF16, name="t2", tag="t2")
            nc.sync.dma_start(th1s_sb[:], th1s[:])
            nc.sync.dma_start(t2_sb[:], t2t[:])
            bia_sb = pw.tile([1, 4 * DIM], BF16, name="bia", tag="bia")
            nc.sync.dma_start(bia_sb[:], bia[:])
            ones_sb = pw.tile([1, DIM], BF16, name="ones", tag="ones")
            nc.any.memset(ones_sb[:], 1.0)

            # persistent activations: th1-fused interleaved Qhat, K^T, V
            # qhat[rc][p=(h,d) in chunk rc, col (j, n16, g)] = th1[g,h] * Q^T[(h,d), n]
            #   with n = 16*j + n16 (local query index)
            qhat_sb = [pw.tile([128, NLOC * 8], BF16, name=f"qh{i}", tag=f"qh{i}")
                       for i in range(4)]
            kt_sb = [pw.tile([128, N], BF16, name=f"kt{i}", tag=f"kt{i}") for i in range(4)]
            v_sb = pw.tile([128, MT, DIM], BF16, name="v", tag="v")

            def evict(dst, src, use_act):
                if use_act:
                    nc.scalar.copy(dst, src)
                else:
                    nc.vector.tensor_copy(dst, src)

            # ================= phase A: x^T + QKV projection =================
            with tc.tile_pool(name="pxt", bufs=1) as pxt:
                qt_sb = [pxt.tile([128, NLOC], BF16, name=f"qt{i}", tag=f"qt{i}")
                         for i in range(4)]
                wq_sb = [pxt.tile([128, DIM], BF16, name=f"wq{i}", tag=f"wq{i}") for i in range(4)]
                wk_sb = [pxt.tile([128, DIM], BF16, name=f"wk{i}", tag=f"wk{i}") for i in range(4)]
                wv_sb = [pxt.tile([128, DIM], BF16, name=f"wv{i}", tag=f"wv{i}") for i in range(4)]
                for i in range(4):
                    nc.sync.dma_start(wq_sb[i][:], wq[128 * i:128 * (i + 1), :])
                    nc.sync.dma_start(wk_sb[i][:], wk[128 * i:128 * (i + 1), :])
                    nc.sync.dma_start(wv_sb[i][:], wv[128 * i:128 * (i + 1), :])

                # x -> x^T (bf16): HWDGE fp32 load, engine cast, xbar transpose
                # xt_sb[p, t, j, f]: dim = j*128+p, key row m = t*128+f
                xt_sb = pxt.tile([128, MT, 4, 128], BF16, name="xt", tag="xt")
                xqt_sb = pxt.tile([128, NT, 4, 128], BF16, name="xqt", tag="xqt")
                for t in range(MT + NT):
                    xb = pxt.tile([128, DIM], BF16, name="xb", tag="xb", bufs=4)
                    if t < MT:
                        nc.gpsimd.dma_start(xb[:], x[128 * t:128 * (t + 1), :])
                        nc.sync.dma_start_transpose(xt_sb[:, t, :, :], xb[:])
                    else:
                        tq = t - MT
                        nc.gpsimd.dma_start(xb[:], xq[128 * tq:128 * (tq + 1), :])
                        nc.sync.dma_start_transpose(xqt_sb[:, tq, :, :], xb[:])

                ei = 0
                for rc in range(4):            # K^T over all keys (needed first)
                    for mch in range(N // 512):
                        ps = psB.tile([128, 512], F32, name="psq", tag="pss")
                        for j in range(4):
                            rhs = xt_sb[:, 4 * mch:4 * (mch + 1), j, :]
                            nc.tensor.matmul(ps[:], wk_sb[j][:, 128 * rc:128 * (rc + 1)],
                                             rhs, start=(j == 0), stop=False)
                        nc.tensor.matmul(ps[:], bia_sb[0:1, 1 * DIM + 128 * rc:1 * DIM + 128 * (rc + 1)],
                                         ones_sb[:, 0:512], start=False, stop=True)
                        evict(kt_sb[rc][:, 512 * mch:512 * (mch + 1)], ps[:], ei % 2 == 0)
                        ei += 1
                for rc in range(4):            # Q^T own half, Qhat build interleaved
                    for nch in range(NLOC // 512):
                        ps = psB.tile([128, 512], F32, name="psq", tag="pss")
                        for j in range(4):
                            rhs = xqt_sb[:, 4 * nch:4 * (nch + 1), j, :]
                            nc.tensor.matmul(ps[:], wq_sb[j][:, 128 * rc:128 * (rc + 1)],
                                             rhs, start=(j == 0), stop=False)
                        nc.tensor.matmul(ps[:], bia_sb[0:1, 0 * DIM + 128 * rc:0 * DIM + 128 * (rc + 1)],
                                         ones_sb[:, 0:512], start=False, stop=True)
                        evict(qt_sb[rc][:, 512 * nch:512 * (nch + 1)], ps[:], ei % 2 == 0)
                        ei += 1
                    qhr = qhat_sb[rc][:].rearrange("p (j n g) -> p j n g",
                                                   n=NG, g=8)
                    qtr = qt_sb[rc][:].rearrange("p (j n) -> p j n", n=NG)
                    for g in range(8):
                        nc.vector.tensor_scalar_mul(
                            qhr[:, :, :, g], qtr, th1s_sb[:, 8 * rc + g:8 * rc + g + 1])
                def vproj(mt):
                    ps = psB.tile([128, 512], F32, name="psq", tag="pss")
                    for j in range(4):
                        nc.tensor.matmul(ps[:], xt_sb[:, mt, j, :], wv_sb[j][:],
                                         start=(j == 0), stop=False)
                    nc.tensor.matmul(ps[:], ones_sb[:, 0:128],
                                     bia_sb[0:1, 2 * DIM:3 * DIM],
                                     start=False, stop=True)
                    evict(v_sb[:, mt, :], ps[:], mt % 2 == 0)
                for mt in range(MT):
                    vproj(mt)
                vfill = []

            # ================= phase B: attention main loop =================
            # Fused design: th1 mix is folded into the score matmuls (Qhat
            # columns are th1-scaled, head-interleaved), so scores land
            # directly in interleaved PSUM: no raw-score evict, no pack DMA.
            # Pipeline: iteration i emits fscores+exp(i) woven with mix2(i-?)
            # and attn@V(i-1): PE bubbles while exp drains are filled by @V.
            with tc.tile_pool(name="pk", bufs=1) as pk:
                st = {}

                def mk_fscore(t):
                    st[t] = {"u": {}, "zz": {},
                             "at": [pk.tile([128, 8, 8, 128], BF16, name=f"at{i}",
                                            tag="at", bufs=3) for i in range(2)]}

                    def fscore(j):
                        u = pk.tile([128, N], BF16, name="u", tag="u", bufs=3)
                        zz = pk.tile([128, 68], F32, name="zz", tag="zz", bufs=3)
                        st[t]["u"][j] = u
                        st[t]["zz"][j] = zz
                        for half in range(2):
                            ps = psA.tile([128, 1024], F32, name="psm1", tag="psm")
                            for mc in range(2):
                                m0 = 1024 * half + 512 * mc
                                for rc in range(4):
                                    nc.tensor.matmul(
                                        ps[:, 512 * mc:512 * (mc + 1)],
                                        qhat_sb[rc][:, 128 * (8 * t + j):
                                                    128 * (8 * t + j) + 128],
                                        kt_sb[rc][:, m0:m0 + 512],
                                        start=(rc == 0), stop=(rc == 3))
                            nc.scalar.activation(u[:, 1024 * half:1024 * (half + 1)],
                                                 ps[:], AF.Exp,
                                                 accum_out=zz[:, half:half + 1])
                    return fscore

                def mk_m2(t):
                    at_h = st[t]["at"]

                    def m2(j):
                        u = st[t]["u"][j]
                        zz = st[t]["zz"][j]
                        nc.vector.tensor_add(zz[:, 2:3], zz[:, 0:1], zz[:, 1:2])
                        nc.vector.reciprocal(zz[:, 3:4], zz[:, 2:3])
                        l2 = zz[:, 4:68].bitcast(BF16)
                        nc.vector.tensor_scalar_mul(l2, t2_sb[:], zz[:, 3:4])
                        a = pk.tile([128, N], BF16, name="a", tag="a", bufs=2)
                        for half in range(2):
                            ps = psA.tile([128, 1024], F32, name="psm2", tag="psm")
                            for mc in range(2):
                                m0 = 1024 * half + 512 * mc
                                nc.tensor.matmul(ps[:, 512 * mc:512 * (mc + 1)],
                                                 l2, u[:, m0:m0 + 512],
                                                 start=True, stop=True)
                            evict(a[:, 1024 * half:1024 * (half + 1)], ps[:],
                                  (2 * j + half) % 4 == 0)
                        for half in range(2):
                            nc.sync.dma_start_transpose(
                                at_h[half][:, j, :, :],
                                a[:, 1024 * half:1024 * (half + 1)])
                    return m2

                def mk_av(tv):
                    at_h = st[tv]["at"]
                    otb = pk.tile([128, 4, 128], BF16, name="otb", tag="otb", bufs=2)
                    st[tv]["otb"] = otb

                    def av(rc):
                        # the two heads of the pair run in the two 64-col
                        # halves of the PE array concurrently (col tiling)
                        ps = psB.tile([128, 128], F32, name="psv", tag="pss")
                        for half in range(2):
                            for mc in range(8):
                                mchunk = half * 8 + mc
                                for gi in range(2):
                                    g = 2 * rc + gi
                                    rhs = at_h[half][:, :, mc, :].rearrange(
                                        "p j (n g) -> p j n g", g=8)[:, :, :, g]
                                    nc.tensor.matmul(
                                        ps[64 * gi:64 * (gi + 1), :],
                                        v_sb[:, mchunk, 64 * g:64 * (g + 1)],
                                        rhs, start=(mchunk == 0),
                                        stop=(mchunk == 15),
                                        tile_position=(0, 64 * gi))
                        nc.vector.tensor_copy(otb[:, rc, :], ps[:])
                    return av

                def emit_outproj(tv):
                    otb = st[tv]["otb"]
                    ps = psB.tile([128, DIM], F32, name="pso", tag="pss")
                    for rc in range(4):
                        nc.tensor.matmul(ps[:], otb[:, rc, :], wo_sb[rc][:],
                                         start=(rc == 0), stop=False)
                    nc.tensor.matmul(ps[:], ones_sb[:, 0:128],
                                     bia_sb[0:1, 3 * DIM:4 * DIM],
                                     start=False, stop=True)
                    yt = pk.tile([128, DIM], F32, name="yt", tag="yt", bufs=2)
                    nc.vector.tensor_copy(yt[:], ps[:])
                    nc.sync.dma_start(y[128 * tv:128 * (tv + 1), :], yt[:])
                    del st[tv]

                for i in range(NT + 1):
                    tm = i if i < NT else None
                    tv = i - 1 if i >= 1 else None
                    fscore = mk_fscore(tm) if tm is not None else None
                    m2 = mk_m2(tm) if tm is not None else None
                    av = mk_av(tv) if tv is not None else None
                    if tm is not None:
                        fscore(0)
                        fscore(1)
                    avq = [0, 1, 2, 3] if tv is not None else []
                    for k in range(8):
                        if tm is not None:
                            m2(k)
                            if k + 2 < 8:
                                fscore(k + 2)
                        if avq and k % 2 == 1:
                            av(avq.pop(0))
                        elif vfill and tv is None:
                            vfill.pop(0)(); vfill.pop(0)()
                            if vfill and k % 2 == 1:
                                vfill.pop(0)()
                    while avq:
                        av(avq.pop(0))
                    if tv is not None:
                        emit_outproj(tv)

    nc.compile()
    return nc


_NC_CACHE = None


def _get_nc():
    global _NC_CACHE
    if _NC_CACHE is None:
        _NC_CACHE = build_nc()
    return _NC_CACHE


def _host_prep(w_qkv, b_qkv, th1, th2, w_out, b_out):
    bf = ml_dtypes.bfloat16
    scale = DH ** -0.5
    w_qkv = np.asarray(w_qkv, dtype=np.float32)
    wq = (w_qkv[:, 0:DIM] * scale).astype(bf)
    wk = w_qkv[:, DIM:2 * DIM].astype(bf)
    wv = w_qkv[:, 2 * DIM:3 * DIM].astype(bf)
    wo = np.asarray(w_out, dtype=np.float32).astype(bf)
    th1 = np.asarray(th1, dtype=np.float32)
    th2 = np.asarray(th2, dtype=np.float32)
    # th1 spread for fused scores: th1s[p, rc*8+g] = th1[g, rc*2 + p//64]
    th1s = np.zeros((128, 32), dtype=np.float32)
    for rc in range(4):
        for g in range(8):
            for p in range(128):
                th1s[p, 8 * rc + g] = th1[g, rc * 2 + p // 64]
    # block-diag template for mix2: T[(n16,h),(n16,g)] = th2[g,h]
    t2t = np.zeros((128, 128), dtype=np.float32)
    for n16 in range(NG):
        t2t[n16 * 8:n16 * 8 + 8, n16 * 8:n16 * 8 + 8] = th2.T
    bqkv = np.asarray(b_qkv, dtype=np.float32)
    bia = np.zeros((1, 4 * DIM), dtype=np.float32)
    bia[0, 0:DIM] = bqkv[0:DIM] * scale     # q bias scaled with w_q
    bia[0, DIM:3 * DIM] = bqkv[DIM:3 * DIM]
    bia[0, 3 * DIM:] = np.asarray(b_out, dtype=np.float32)
    return (wq, wk, wv, wo, th1s, t2t.astype(bf), bia.astype(bf))


def kernel(x, w_qkv, b_qkv, th1, th2, w_out, b_out):
    x = np.asarray(x, dtype=np.float32)
    wq, wk, wv, wo, th1s, t2t, bia = _host_prep(w_qkv, b_qkv, th1, th2, w_out, b_out)
    nc = _get_nc()
    in_maps = []
    for c in range(NCORES):
        b, half = c // 2, c % 2
        in_maps.append({
            "x": np.ascontiguousarray(x[b]),
            "xq": np.ascontiguousarray(x[b, NLOC * half:NLOC * (half + 1), :]),
            "wq": wq, "wk": wk, "wv": wv, "wo": wo,
            "th1s": th1s, "t2t": t2t, "bia": bia,
        })
    res = run_bass_kernel_spmd(nc, in_maps, core_ids=list(range(NCORES)))
    out = np.empty((B, N, DIM), dtype=np.float32)
    for c in range(NCORES):
        b, half = c // 2, c % 2
        out[b, NLOC * half:NLOC * (half + 1), :] = res.results[c]["y"]
    return out
